# revision 28
# baseline (speedup 1.0000x reference)
"""Two-layer GAT on 8 Trainium2 NeuronCores — dense-edge-table design.

The previous gather-based kernel was GpSimd-bound: dma_gather descriptor
generation on the Q7 cores cost ~35us per 1024-index call (~9.7ms of Q7 work
per layer).  This version removes every indexed DMA from the device.  The
host (which already owns the edge sort / shard step) expands the per-node
table into a dense per-edge table between launches; the device then runs
pure sequential DMA + compute:

  B-launch (per layer, nodes sharded N/8 per core):
      h_ext[n] = [ g = x@W (256 cols, head-interleaved) | as | ad ]  (fp16)
      via PE matmuls with waug = [W | W@As | W@Ad] (assembled on device).
  host: expand h_ext rows per edge into a dst-partitioned dense layout:
      slot (block k, partition p, t) holds edge t of dst node perm[k,p]:
      row = [ g[src] | as[src] | ad[dst] ]; dummy slots get alphas=-30000
      so exp(leakyrelu(as+ad)) underflows to exactly 0.
  E-launch (per layer, dst nodes sharded by block):
      ex = exp(leakyrelu(as+ad)); rhs = ex (x) g in bf16 (DVE 2x mode);
      num-reduce over t = identity-matmul accumulation into PSUM (PE,
      pairs of tiles per 512-col matmul); den-reduce on DVE;
      out = gelu(num/den)  [bias pre-folded into g in the B-launch].

Blocks are degree-homogeneous (nodes sorted by degree, consecutive groups of
128) so tiles-per-block ~= mean degree; the per-rank tile count Tk is shared
by all 8 cores (SPMD shape uniformity) with ~2.5% padding.
"""
import sys
sys.path.insert(0, '/opt/trn_rl_repo')
import numpy as np
from concourse import bass, bacc, tile, mybir
from concourse.bass_utils import run_bass_kernel_spmd

F16 = mybir.dt.float16
BF16 = mybir.dt.bfloat16
F32 = mybir.dt.float32

N, D, H, C = 50000, 256, 4, 64
NCORES = 8
NPAD = 50176            # 392 blocks of 128
NBLK = 49               # blocks per core
SLAB = NPAD // NCORES   # 6272 node rows per core in B-launch
EXT = 264               # g(256) | as(4) | ad(4)
GRP = 32                # tiles per processing group in E-launch
FINB = 8                # blocks per gelu/output batch in E-launch


# ----------------------------------------------------------------- host plan
def make_plan(edge_index):
    src = np.asarray(edge_index[0], dtype=np.int64)
    dst = np.asarray(edge_index[1], dtype=np.int64)
    loops = np.arange(N, dtype=np.int64)
    src = np.concatenate([src, loops])
    dst = np.concatenate([dst, loops])

    deg = np.bincount(dst, minlength=N)  # includes self loop
    degp = np.zeros(NPAD, dtype=np.int64)
    degp[:N] = deg
    order = np.argsort(-degp, kind='stable')  # node ids, degree desc

    # group g (0..391) = nodes order[128g:128g+128]; serpentine deal to cores
    ngrp = NPAD // 128
    grp_core = np.empty(ngrp, dtype=np.int64)
    grp_rank = np.empty(ngrp, dtype=np.int64)
    for g in range(ngrp):
        rnd, pos = divmod(g, NCORES)
        grp_core[g] = pos if rnd % 2 == 0 else NCORES - 1 - pos
        grp_rank[g] = rnd
    Tg = degp[order].reshape(ngrp, 128).max(axis=1)
    Tk = np.zeros(NBLK, dtype=np.int64)
    for g in range(ngrp):
        Tk[grp_rank[g]] = max(Tk[grp_rank[g]], Tg[g])
    Tk = np.maximum(Tk, 1)

    # perm[core, k, p] = node id (or >=N for pad nodes)
    perm = np.empty((NCORES, NBLK, 128), dtype=np.int64)
    for g in range(ngrp):
        perm[grp_core[g], grp_rank[g]] = order[g * 128:(g + 1) * 128]


    # per-node placement
    core_of = np.empty(NPAD, dtype=np.int64)
    k_of = np.empty(NPAD, dtype=np.int64)
    p_of = np.empty(NPAD, dtype=np.int64)
    cc, kk, pp = np.meshgrid(np.arange(NCORES), np.arange(NBLK),
                             np.arange(128), indexing='ij')
    core_of[perm.ravel()] = cc.ravel()
    k_of[perm.ravel()] = kk.ravel()
    p_of[perm.ravel()] = pp.ravel()

    # flat slot base per block: rows for block k span [boff[k]*128, +128*Tk[k])
    # within a block rows are p-major: flat = boff[k]*128 + p*Tk[k] + t
    boff = np.zeros(NBLK + 1, dtype=np.int64)
    boff[1:] = np.cumsum(Tk)
    NT = int(boff[-1])
    R = NT * 128

    # edges sorted by dst; position within dst run
    eorder = np.argsort(dst, kind='stable')
    dsts = dst[eorder]
    srcs = src[eorder]
    starts = np.searchsorted(dsts, np.arange(N + 1))
    q = np.arange(len(dsts)) - starts[dsts]

    flat = boff[k_of[dsts]] * 128 + p_of[dsts] * Tk[k_of[dsts]] + q
    srcv, dstv = [], []
    for c in range(NCORES):
        sv = np.full(R, -1, dtype=np.int64)
        m = core_of[dsts] == c
        sv[flat[m]] = srcs[m]
        srcv.append(sv)
        # dst node of each slot (p-major repeat of perm rows)
        dv = np.concatenate([np.repeat(perm[c, k], Tk[k])
                             for k in range(NBLK)])
        dstv.append(dv)
    return dict(Tk=Tk, perm=perm, NT=NT, R=R, srcv=srcv, dstv=dstv)


def interleave_cols(M, axis=-1):
    """reorder feature axis from (h,c)->h*C+c to (c,h)->c*H+h."""
    M = np.moveaxis(M, axis, -1)
    sh = M.shape
    M = M.reshape(sh[:-1] + (H, C)).swapaxes(-1, -2).reshape(sh)
    return np.moveaxis(M, -1, axis)


def deinterleave_cols(M, axis=-1):
    M = np.moveaxis(M, axis, -1)
    sh = M.shape
    M = M.reshape(sh[:-1] + (C, H)).swapaxes(-1, -2).reshape(sh)
    return np.moveaxis(M, -1, axis)


def weight_inputs(W, a_s, a_d, b):
    """Per-layer weight arrays for the B-launch (baseline Phase-A layout)."""
    Wi = interleave_cols(np.asarray(W, np.float32), axis=1).astype(np.float16)
    WTf = np.ascontiguousarray(Wi.T)  # [f_out interleaved, f_in] fp16
    a_s = np.asarray(a_s, np.float32)
    a_d = np.asarray(a_d, np.float32)
    Amat = np.zeros((2, 128, 8), dtype=np.float16)
    for hh in range(2):
        rows = np.arange(hh * 128, (hh + 1) * 128)
        c_, h_ = rows // H, rows % H
        Amat[hh, np.arange(128), h_] = a_s[h_, c_]
        Amat[hh, np.arange(128), 4 + h_] = a_d[h_, c_]
    # bias folded into the B-launch g columns: softmax weights sum to 1, so
    # sum_e alpha_e (g+b) = sum_e alpha_e g + b exactly; alpha cols get 0.
    bias = np.zeros((128, EXT), dtype=np.float32)
    bias[:, 0:256] = interleave_cols(
        np.asarray(b, np.float32).reshape(1, 256), axis=1)
    return dict(W=Wi, WT=WTf, Amat=Amat, bias=bias)


# ------------------------------------------------------------------ B kernel
def build_b_kernel():
    """h_ext slab: [SLAB, 264] fp16 = [x@W | as | ad] for one N/8 node slab."""
    nc = bacc.Bacc("TRN2", target_bir_lowering=False, debug=False,
                   num_devices=NCORES)
    xT = nc.declare_dram_parameter("xT", [256, SLAB], F16, isOutput=False)
    Wp = nc.declare_dram_parameter("W", [256, 256], F16, isOutput=False)
    WTp = nc.declare_dram_parameter("WT", [256, 256], F16, isOutput=False)
    Ap = nc.declare_dram_parameter("Amat", [2, 128, 8], F16, isOutput=False)
    Bp = nc.declare_dram_parameter("bias", [128, EXT], F32, isOutput=False)
    hout = nc.declare_dram_parameter("hext", [SLAB, EXT], F16, isOutput=True)
    NRT = SLAB // 128

    with tile.TileContext(nc) as tc:
        with (
            tc.tile_pool(name="const", bufs=1) as constp,
            tc.tile_pool(name="mm", bufs=3) as mmp,
            tc.tile_pool(name="psum", bufs=4, space="PSUM") as pp,
            tc.tile_pool(name="psumw", bufs=1, space="PSUM") as ppw,
        ):
            # waug = [W | W@As | W@Ad]   [128, 2, 264]
            waug = constp.tile([128, 2, EXT], F16)
            for kh in range(2):
                nc.sync.dma_start(out=waug[:, kh, 0:256],
                                  in_=Wp[kh * 128:(kh + 1) * 128, :])
            wts = constp.tile([128, 2, 256], F16)
            for ho in range(2):
                nc.sync.dma_start(out=wts[:, ho, :],
                                  in_=WTp[ho * 128:(ho + 1) * 128, :])
            amat = constp.tile([128, 2, 8], F16)
            nc.sync.dma_start(out=amat[:],
                              in_=Ap[:, :, :].rearrange("g p d -> p g d"))
            biast = constp.tile([128, EXT], F32)
            nc.sync.dma_start(out=biast[:], in_=Bp[:, :])
            for hi in range(2):
                pw = ppw.tile([128, 8], F32)
                for ho in range(2):
                    nc.tensor.matmul(pw[:], wts[:, ho, hi * 128:(hi + 1) * 128],
                                     amat[:, ho, :], start=(ho == 0),
                                     stop=(ho == 1))
                nc.vector.tensor_copy(waug[:, hi, 256:264], pw[:])

            XSL = 17
            for s0 in range(0, NRT, XSL):
                ntile = min(XSL, NRT - s0)
                xsl = mmp.tile([128, 2, XSL * 128], F16, tag="xsl")
                for kh in range(2):
                    nc.sync.dma_start(
                        out=xsl[:, kh, 0:ntile * 128],
                        in_=xT[kh * 128:(kh + 1) * 128,
                               s0 * 128:(s0 + ntile) * 128])
                gtile = mmp.tile([128, XSL, EXT], F16, tag="gw")
                for t in range(ntile):
                    ps = pp.tile([128, EXT], F32, tag="psB")
                    for kh in range(2):
                        nc.tensor.matmul(ps[:],
                                         xsl[:, kh, t * 128:(t + 1) * 128],
                                         waug[:, kh, :],
                                         start=(kh == 0), stop=(kh == 1))
                    nc.vector.tensor_tensor(out=gtile[:, t, :], in0=ps[:],
                                            in1=biast[:],
                                            op=mybir.AluOpType.add)
                nc.sync.dma_start(
                    out=hout[s0 * 128:(s0 + ntile) * 128, :].rearrange(
                        "(b p) f -> p b f", p=128),
                    in_=gtile[:, 0:ntile, :])
    nc.compile()
    return nc


# --------------------------------------------------------------- edge kernel
def build_edge_kernel(Tk):
    Tk = [int(t) for t in Tk]
    NT = sum(Tk)
    R = NT * 128
    DBL = NBLK * 128
    nc = bacc.Bacc("TRN2", target_bir_lowering=False, debug=False,
                   num_devices=NCORES)
    ER = nc.declare_dram_parameter("edge_rows", [R, EXT], F16, isOutput=False)
    Ip = nc.declare_dram_parameter("ident", [128, 128], BF16, isOutput=False)
    out = nc.declare_dram_parameter("out", [DBL, 256], F32, isOutput=True)

    with tile.TileContext(nc) as tc:
        with (
            tc.tile_pool(name="const", bufs=1) as constp,
            tc.tile_pool(name="g", bufs=3) as gp,
            tc.tile_pool(name="r", bufs=2) as rp,
            tc.tile_pool(name="ew", bufs=3) as ewp,
            tc.tile_pool(name="fin", bufs=2) as fp_,
            tc.tile_pool(name="psum", bufs=6, space="PSUM") as pp,
        ):
            ident = constp.tile([128, 128], BF16)
            nc.sync.dma_start(out=ident[:], in_=Ip[:, :])

            boff = 0
            finbuf = None
            for k in range(NBLK):
                T = Tk[k]
                kb = k % FINB
                if kb == 0:
                    finbuf = fp_.tile([128, FINB, 256], F32, tag="finbuf")
                dent = ewp.tile([128, 4], F32, tag="den")
                den = dent[:]
                blk = ER[boff * 128:(boff + T) * 128, :].rearrange(
                    "(p t) f -> p t f", p=128)
                ps = pp.tile([128, 512], F32, tag="ps")
                mm_started = False
                for g0 in range(0, T, GRP):
                    gsz = min(GRP, T - g0)
                    gt = gp.tile([128, GRP, EXT], F16, tag="gt")
                    nc.sync.dma_start(out=gt[:, 0:gsz, :],
                                      in_=blk[:, g0:g0 + gsz, :])
                    tt = ewp.tile([128, GRP, 4], F32, tag="tt")
                    nc.gpsimd.tensor_tensor(out=tt[:, 0:gsz, :],
                                            in0=gt[:, 0:gsz, 256:260],
                                            in1=gt[:, 0:gsz, 260:264],
                                            op=mybir.AluOpType.add)
                    ut = ewp.tile([128, GRP, 4], F32, tag="ut")
                    nc.vector.scalar_tensor_tensor(
                        out=ut[:, 0:gsz, :], in0=tt[:, 0:gsz, :], scalar=0.2,
                        in1=tt[:, 0:gsz, :], op0=mybir.AluOpType.mult,
                        op1=mybir.AluOpType.max)
                    exf = ewp.tile([128, GRP, 4], F16, tag="ex")
                    nc.scalar.activation(out=exf[:, 0:gsz, :],
                                         in_=ut[:, 0:gsz, :],
                                         func=mybir.ActivationFunctionType.Exp)
                    # den partial (sum over t): view [p, h, t], reduce inner
                    if g0 == 0:
                        nc.vector.tensor_reduce(
                            out=den, in_=exf[:, 0:gsz, :].rearrange(
                                "p t h -> p h t"),
                            axis=mybir.AxisListType.X, op=mybir.AluOpType.add)
                    else:
                        dtmp = ewp.tile([128, 4], F32, tag="dt")
                        nc.vector.tensor_reduce(
                            out=dtmp[:], in_=exf[:, 0:gsz, :].rearrange(
                                "p t h -> p h t"),
                            axis=mybir.AxisListType.X, op=mybir.AluOpType.add)
                        nc.vector.tensor_tensor(out=den, in0=den,
                                                in1=dtmp[:],
                                                op=mybir.AluOpType.add)
                    # rhs = ex (x) g  in bf16
                    rhs = rp.tile([128, GRP, 256], BF16, tag="rhs")
                    nc.vector.tensor_tensor(
                        out=rhs[:, 0:gsz, :].rearrange(
                            "p t (c h) -> p t c h", h=4),
                        in0=gt[:, 0:gsz, 0:256].rearrange(
                            "p t (c h) -> p t c h", h=4),
                        in1=exf[:, 0:gsz, :].unsqueeze(2).broadcast_to(
                            [128, gsz, 64, 4]),
                        op=mybir.AluOpType.mult)
                    npair = gsz
                    if gsz % 2:
                        nc.gpsimd.memset(rhs[:, gsz, :], 0)
                        npair += 1
                    for j in range(0, npair, 2):
                        nc.tensor.matmul(
                            ps[:, :],
                            ident[:, :],
                            rhs[:, j:j + 2, :].rearrange("p t f -> p (t f)"),
                            start=(not mm_started),
                            stop=(g0 + GRP >= T and j + 2 >= npair))
                        mm_started = True
                # per-block: normalize num into finbuf slot (DVE only)
                rinv = ewp.tile([128, 4], F32, tag="rinv")
                nc.vector.reciprocal(rinv[:], den)
                fin = finbuf[:, kb, :]
                nc.scalar.copy(out=fin, in_=ps[:, 0:256])
                nc.vector.tensor_tensor(out=fin, in0=fin,
                                        in1=ps[:, 256:512],
                                        op=mybir.AluOpType.add)
                nc.vector.tensor_tensor(
                    out=fin.rearrange("p (c h) -> p c h", h=4),
                    in0=fin.rearrange("p (c h) -> p c h", h=4),
                    in1=rinv[:].unsqueeze(1).broadcast_to([128, 64, 4]),
                    op=mybir.AluOpType.mult)
                if kb == FINB - 1 or k == NBLK - 1:
                    nb = kb + 1
                    k0 = k - kb
                    nc.scalar.activation(out=finbuf[:, 0:nb, :],
                                         in_=finbuf[:, 0:nb, :],
                                         func=mybir.ActivationFunctionType.Gelu)
                    nc.sync.dma_start(
                        out=out[k0 * 128:(k0 + nb) * 128, :].rearrange(
                            "(b p) f -> p b f", p=128),
                        in_=finbuf[:, 0:nb, :])
                boff += T
    nc.compile()
    return nc


# ------------------------------------------------------------------ host ops
def expand(plan, h_ext):
    """h_ext [NPAD, EXT] fp16 -> per-core dense edge tables [R, EXT] fp16."""
    tables = []
    for c in range(NCORES):
        sv, dv = plan['srcv'][c], plan['dstv'][c]
        rows = h_ext[np.maximum(sv, 0)]
        rows[:, 260:264] = h_ext[dv, 260:264]
        rows[sv < 0, 256:264] = -30000.0
        tables.append(rows)
    return tables


def assemble(plan, outs):
    """per-core [NBLK*128, 256] fp32 (interleaved) -> [N, 256] natural fp32."""
    full = np.zeros((NPAD, 256), dtype=np.float32)
    for c in range(NCORES):
        full[plan['perm'][c].reshape(-1)] = outs[c]
    return deinterleave_cols(full[:N], axis=1)


def run_b(nc_b, x_nat, winp, run):
    """x_nat [N,256] fp32 natural cols -> h_ext [NPAD, EXT] fp16."""
    xp = np.zeros((NPAD, 256), dtype=np.float16)
    xp[:N] = x_nat.astype(np.float16)
    xT = np.ascontiguousarray(xp.T)
    in_maps = []
    for c in range(NCORES):
        in_maps.append(dict(xT=np.ascontiguousarray(
            xT[:, c * SLAB:(c + 1) * SLAB]),
            W=winp['W'], WT=winp['WT'], Amat=winp['Amat'],
            bias=winp['bias']))
    r = run(nc_b, in_maps)
    return np.concatenate([m['hext'] for m in r.results], axis=0)


def run_e(nc_e, plan, tables, ident, run):
    in_maps = []
    for c in range(NCORES):
        in_maps.append(dict(edge_rows=tables[c], ident=ident))
    r = run(nc_e, in_maps)
    return [m['out'] for m in r.results]


def gat_forward(x, edge_index, W0, a_s0, a_d0, b0, W1, a_s1, a_d1, b1, run):
    plan = make_plan(edge_index)
    nc_b = build_b_kernel()
    nc_e = build_edge_kernel(plan['Tk'])
    import ml_dtypes
    ident = np.eye(128, dtype=ml_dtypes.bfloat16)

    w0 = weight_inputs(W0, a_s0, a_d0, b0)
    w1 = weight_inputs(W1, a_s1, a_d1, b1)

    hx0 = run_b(nc_b, np.asarray(x, np.float32), w0, run)
    t0 = expand(plan, hx0)
    o0 = run_e(nc_e, plan, t0, ident, run)
    h1 = assemble(plan, o0)

    hx1 = run_b(nc_b, h1, w1, run)
    t1 = expand(plan, hx1)
    o1 = run_e(nc_e, plan, t1, ident, run)
    return assemble(plan, o1)


# ------------------------------------------------------------- harness entry
def kernel(x, edge_index, edge_attr=None, W0=None, a_src0=None, a_dst0=None,
           b0=None, W1=None, a_src1=None, a_dst1=None, b1=None):
    def run(nc, in_maps):
        return run_bass_kernel_spmd(nc, in_maps, list(range(NCORES)))
    out = gat_forward(np.asarray(x), np.asarray(edge_index),
                      np.asarray(W0), np.asarray(a_src0), np.asarray(a_dst0),
                      np.asarray(b0), np.asarray(W1), np.asarray(a_src1),
                      np.asarray(a_dst1), np.asarray(b1), run)
    return out.astype(np.float32)
